# revision 6
# baseline (speedup 1.0000x reference)
"""Multi-head attention TRN2 Bass kernel (8 NeuronCores, SPMD).

Problem: B=4, S=1024, E=1024, H=16 heads of dim 64, fp32.
    Q = q @ Wq^T (per head), K, V likewise
    scores = Q K^T / 8 ; P = softmax(scores) ; ctx = P V
    out = concat_heads(ctx) @ Wo^T

Sharding: core c handles batch b = c // 2 and head-group g = c % 2
(8 heads each). Each core computes a partial output projection over its
512 concat features; the host sums the two partials per batch.

Schedule (v3): engines have in-order queues, so emission order IS the
schedule. The kernel is one hand-woven software pipeline built from a
filler-unit queue: each attention block emits its 8 score-matmul/exp
slots and pulls ~1.1us of filler (projections, V projection, deferred
ctx blocks, output projection) between slots, so the PE never sits
head-blocked behind an ACT-paced score matmul and the scalar engine's
73us exp stream is fed from ~18us on.
  - ctx runs one block behind its scores (block j's slots host block
    j-1's ctx matmuls): 16 eAB tiles buffer two blocks, pp_ctx's two
    banks only ever hold one block's accumulators.
  - 6 dummy matmuls on a zeroed scratch tile warm the PE clock-gate
    during the framework preamble; wq/wk pair-0 + the first xq chunk
    bootstrap-load from the vector queue (its preamble ends ~3.5us,
    the sync queue's at ~7us).
  - DMA count is kept low (~20 loads, 8 stores) because every DMA
    semaphore costs ~115ns/engine in the framework epilogue.
  - Output projection: wave0 (rows 0:511, needs only sh=0 cat) slots
    under the sh=1 blocks; wave1 defers every pair-3 contribution so
    24 prep matmuls keep the PE warm+busy through the final normalize
    and only 14 matmuls + split scalar/vector drains trail it.
  - Output is stored bf16 (halves the tail DMA); host sums in fp32.
  - PSUM: pp_sc 2x[128,1024] scores, pp_ctx 2x[128,512] ctx/denom,
    pp_mm 2x[128,512] projections + outproj = 8 banks exactly.
"""

from contextlib import ExitStack

import ml_dtypes
import numpy as np

import concourse.bacc as bacc
import concourse.mybir as mybir
import concourse.tile as tile
from concourse.bass_utils import run_bass_kernel_spmd

B, S, E, H = 4, 1024, 1024, 16
HD = 64          # head dim
HPC = 8          # heads per core
NPAIR = 4        # head pairs per core
NET = 8          # e-tiles (E / 128)
NTT = 8          # t-tiles (S / 128)
P = 128

F32 = mybir.dt.float32
BF16 = mybir.dt.bfloat16
EXP = mybir.ActivationFunctionType.Exp
SCALE = 1.0 / 8.0  # 1/sqrt(HD)
BF = ml_dtypes.bfloat16


class Weave:
    """Ordered filler-unit queue; units = (tag, cost_ns, emit_fn)."""

    def __init__(self):
        self.q = []

    def add(self, tag, cost, fn):
        self.q.append((tag, cost, fn))

    def pull(self, budget):
        spent = 0
        while self.q and spent < budget:
            tag, cost, fn = self.q.pop(0)
            fn()
            spent += cost
        return spent

    def pull_tag(self, tag):
        """Emit every unit up to and including the last one tagged `tag`."""
        idx = max((i for i, u in enumerate(self.q) if u[0] == tag), default=-1)
        for _ in range(idx + 1):
            _, _, fn = self.q.pop(0)
            fn()


def _emit(nc, tc, ctx, aps):
    xqT, xkT, xvT, wqT, wkT, wvT, woT, out = aps

    const = ctx.enter_context(tc.tile_pool(name="const", bufs=1))
    etp = ctx.enter_context(tc.tile_pool(name="etp", bufs=24))
    obp = ctx.enter_context(tc.tile_pool(name="obp", bufs=4))
    rcp = ctx.enter_context(tc.tile_pool(name="rcp", bufs=8))
    pp_mm = ctx.enter_context(tc.tile_pool(name="pp_mm", bufs=2, space="PSUM"))
    pp_sc = ctx.enter_context(tc.tile_pool(name="pp_sc", bufs=2, space="PSUM"))
    pp_ctx = ctx.enter_context(tc.tile_pool(name="pp_ctx", bufs=2, space="PSUM"))

    wo_t = const.tile([P, 4096], BF16, name="wo_t")
    qt = const.tile([P, 4096], BF16, name="qt")
    kt = const.tile([P, 4096], BF16, name="kt")
    vaug = const.tile([P, 8192], BF16, name="vaug")
    cat = const.tile([P, 4096], BF16, name="cat")
    xq = const.tile([P, 8192], BF16, name="xq")
    xk = const.tile([P, 8192], BF16, name="xk")
    xv = const.tile([P, 8192], BF16, name="xv")
    wq = const.tile([P, 4096], BF16, name="wq")
    wk = const.tile([P, 4096], BF16, name="wk")
    wv = const.tile([P, 4096], BF16, name="wv")
    scr = const.tile([P, 512], BF16, name="scr")

    # scratch (DVE: its preamble ends earliest) + V-augmentation ones
    nc.vector.memset(scr[:], 0.0)
    v4 = vaug[:, :].rearrange("p (j q c) -> p j q c", q=2, c=P)
    nc.gpsimd.memset(v4[:, :, 0, HD:P], 1.0)
    nc.gpsimd.memset(v4[:, :, 1, 0:HD], 1.0)

    # ---- PE warm-up: dummy matmuls release the HAM clock gate while
    # the first input DMAs are still in flight ----
    psd = pp_mm.tile([P, 512], F32, name="psd", tag="mm")
    for i in range(6):
        nc.tensor.matmul(psd[:], lhsT=scr[:, 0:P], rhs=scr[:],
                         start=(i == 0), stop=(i == 5))

    # ---- input DMAs. Bootstrap (vector queue, early preamble exit):
    # wq/wk pair 0 + first xq chunk. Rest on sync in priority order. ----
    nc.scalar.dma_start(out=wq[:, 0:1024], in_=wqT[:, 0:1024])
    nc.scalar.dma_start(out=wk[:, 0:1024], in_=wkT[:, 0:1024])
    nc.scalar.dma_start(out=xq[:, 0:2048], in_=xqT[:, 0:2048])
    for c in range(1, 4):
        nc.sync.dma_start(out=xq[:, c * 2048:(c + 1) * 2048],
                          in_=xqT[:, c * 2048:(c + 1) * 2048])
    for c in range(4):
        nc.sync.dma_start(out=xk[:, c * 2048:(c + 1) * 2048],
                          in_=xkT[:, c * 2048:(c + 1) * 2048])
    nc.sync.dma_start(out=wq[:, 1024:4096], in_=wqT[:, 1024:4096])
    nc.sync.dma_start(out=wk[:, 1024:4096], in_=wkT[:, 1024:4096])
    nc.sync.dma_start(out=wv[:], in_=wvT[:])
    for c in range(4):
        nc.sync.dma_start(out=xv[:, c * 2048:(c + 1) * 2048],
                          in_=xvT[:, c * 2048:(c + 1) * 2048])
    nc.sync.dma_start(out=wo_t[:], in_=woT[:])

    # ---- building blocks ----
    def proj_half(w, x, dst, p, sh, half, ps_box):
        # half 0: allocate psum + et 0..3 ; half 1: et 4..7 + drain
        if half == 0:
            ps_box[0] = pp_mm.tile([P, 512], F32, name="ps", tag="mm")
        ps = ps_box[0]
        for et in (range(4) if half == 0 else range(4, 8)):
            nc.tensor.matmul(
                ps[:],
                lhsT=w[:, p * 1024 + et * P:p * 1024 + (et + 1) * P],
                rhs=x[:, et * 1024 + sh * 512:et * 1024 + (sh + 1) * 512],
                start=(et == 0), stop=(et == NET - 1),
            )
        if half == 1:
            nc.vector.tensor_copy(
                dst[:, p * 1024 + sh * 512:p * 1024 + (sh + 1) * 512], ps[:])

    def vproj_tt(tt):
        ps = pp_mm.tile([P, 512], F32, name="psv", tag="mm")
        for et in range(NET):
            nc.tensor.matmul(
                ps[:],
                lhsT=xv[:, et * 1024 + tt * P:et * 1024 + (tt + 1) * P],
                rhs=wv[:, et * 512:(et + 1) * 512],
                start=(et == 0), stop=(et == NET - 1),
            )
        dstt = vaug[:, tt * 1024:(tt + 1) * 1024].rearrange(
            "p (j q c) -> p j q c", q=2, c=P)
        srcv = ps[:].rearrange("p (j q c) -> p j q c", q=2, c=HD)
        nc.vector.tensor_copy(dstt[:, :, 0, 0:HD], srcv[:, :, 0, :])
        nc.vector.tensor_copy(dstt[:, :, 1, HD:P], srcv[:, :, 1, :])

    def scores_tt(sh, p, tt):
        qcol = p * 1024 + sh * 512
        kcol = p * 1024 + tt * P
        sAB = pp_sc.tile([P, 1024], F32, name="sAB", tag="sc")
        nc.tensor.matmul(sAB[:, 0:512],
                         lhsT=kt[0:HD, kcol:kcol + P],
                         rhs=qt[0:HD, qcol:qcol + 512],
                         start=True, stop=True)
        nc.tensor.matmul(sAB[:, 512:1024],
                         lhsT=kt[HD:P, kcol:kcol + P],
                         rhs=qt[HD:P, qcol:qcol + 512],
                         start=True, stop=True)
        eAB = etp.tile([P, 1024], BF16, name="eAB", tag="et")
        nc.scalar.activation(eAB[:], sAB[:], EXP, scale=SCALE)
        return eAB

    def ctx_mms(p, tt, eAB, ctxA, ctxB):
        bA = (tt * HPC + 2 * p) * P
        bB = bA + P
        nc.tensor.matmul(ctxA[:], lhsT=vaug[:, bA:bA + P],
                         rhs=eAB[:, 0:512],
                         start=(tt == 0), stop=(tt == NTT - 1))
        nc.tensor.matmul(ctxB[:], lhsT=vaug[:, bB:bB + P],
                         rhs=eAB[:, 512:1024],
                         start=(tt == 0), stop=(tt == NTT - 1))

    def normalize(ctxA, ctxB, qcol):
        # A: ctx rows 0:64, denom rows 64:128 -> move denom to base 0
        # (reciprocal_approx_fast requirement) via gpsimd-queue DMA.
        # B: mirrored, broadcast the reciprocal up instead.
        rA = rcp.tile([P, 512], F32, name="rA", tag="rc")
        rA2 = rcp.tile([P, 512], F32, name="rA2", tag="rc")
        nc.vector.tensor_copy(rA[HD:P, :], ctxA[HD:P, :])
        nc.gpsimd.dma_start(out=rA[0:HD, :], in_=rA[HD:P, :])
        nc.vector.reciprocal_approx_fast(rA2[0:HD, :], rA[0:HD, :])
        nc.vector.tensor_mul(cat[0:HD, qcol:qcol + 512],
                             ctxA[0:HD, :], rA2[0:HD, :])
        rB = rcp.tile([P, 512], F32, name="rB", tag="rc")
        nc.vector.reciprocal_approx_fast(rB[0:HD, :], ctxB[0:HD, :])
        nc.gpsimd.dma_start(out=rB[HD:P, :], in_=rB[0:HD, :])
        nc.vector.tensor_mul(cat[HD:P, qcol:qcol + 512],
                             ctxB[HD:P, :], rB[HD:P, :])

    def wave0_st(st, store_eng):
        ps_h = []
        for ih in range(2):
            ps = pp_mm.tile([P, 512], F32, name="po", tag="mm")
            ps_h.append(ps)
            for p4 in range(4):
                nc.tensor.matmul(
                    ps[:],
                    lhsT=cat[:, p4 * 1024 + st * P:p4 * 1024 + (st + 1) * P],
                    rhs=wo_t[:, p4 * 1024 + ih * 512:p4 * 1024 + (ih + 1) * 512],
                    start=(p4 == 0), stop=(p4 == 3))
        ob = obp.tile([P, 1024], BF16, name="ob", tag="ob")
        nc.vector.tensor_copy(ob[:, 0:512], ps_h[0][:])
        nc.vector.tensor_copy(ob[:, 512:1024], ps_h[1][:])
        store_eng.dma_start(out=out[st * P:(st + 1) * P, :], in_=ob[:])

    # ---- the pipeline ----
    wv_ = Weave()
    kbox = [None]
    wv_.add('kp0s1', 850, lambda: proj_half(wk, xk, kt, 0, 1, 0, kbox))
    wv_.add('kp0s1', 850, lambda: proj_half(wk, xk, kt, 0, 1, 1, kbox))
    boxes = {}
    for p in (1, 2, 3):
        for wmat, xmat, dst, nm in ((wq, xq, qt, 'q'), (wk, xk, kt, 'k')):
            for sh in range(2):
                bx = boxes[(nm, p, sh)] = [None]
                tg = f'p{p}'
                wv_.add(tg, 850, (lambda w_, x_, d_, p_, s_, b_:
                                  lambda: proj_half(w_, x_, d_, p_, s_, 0, b_))(
                                      wmat, xmat, dst, p, sh, bx))
                wv_.add(tg, 850, (lambda w_, x_, d_, p_, s_, b_:
                                  lambda: proj_half(w_, x_, d_, p_, s_, 1, b_))(
                                      wmat, xmat, dst, p, sh, bx))
    for tt in range(NTT):
        wv_.add('vproj', 1700, (lambda t: lambda: vproj_tt(t))(tt))

    order = [(0, 0), (0, 1), (0, 2), (0, 3), (1, 0), (1, 1), (1, 2), (1, 3)]
    eab_hist = {}
    ctx_done = 0          # blocks whose ctx units have been queued

    def queue_ctx_block(j):
        sh, p = order[j]
        qcol = p * 1024 + sh * 512
        cbox = [None, None]

        def start_ctx(b_=cbox, p_=p, j_=j):
            b_[0] = pp_ctx.tile([P, 512], F32, name="ctxA", tag="ctx")
            b_[1] = pp_ctx.tile([P, 512], F32, name="ctxB", tag="ctx")
            for t in (0, 1):
                ctx_mms(p_, t, eab_hist[j_][t], b_[0], b_[1])
        wv_.add(f'ctx{j}', 850, start_ctx)
        for t0 in (2, 4, 6):
            wv_.add(f'ctx{j}', 850,
                    (lambda ts, b_=cbox, p_=p, j_=j:
                     lambda: [ctx_mms(p_, t, eab_hist[j_][t], b_[0], b_[1])
                              for t in ts])((t0, t0 + 1)))
        wv_.add(f'ctx{j}', 200,
                (lambda b_=cbox, q_=qcol: lambda: normalize(b_[0], b_[1], q_))())

    # pair-0 Q projection + K sh0 run directly (nothing else is ready)
    bq0, bq1, bk0 = [None], [None], [None]
    proj_half(wq, xq, qt, 0, 0, 0, bq0)
    proj_half(wq, xq, qt, 0, 0, 1, bq0)
    proj_half(wq, xq, qt, 0, 1, 0, bq1)
    proj_half(wq, xq, qt, 0, 1, 1, bq1)
    proj_half(wk, xk, kt, 0, 0, 0, bk0)
    proj_half(wk, xk, kt, 0, 0, 1, bk0)

    for j, (sh, p) in enumerate(order):
        # hard prerequisites for this block's scores
        if j == 0:
            pass
        elif j <= 3:
            wv_.pull_tag(f'p{p}')
        if j >= 3:
            wv_.pull_tag(f'ctx{j - 3}')       # frees this block's eAB slots
        eabs = []
        for tt in range(NTT):
            if j == 0 and tt == 4:
                wv_.pull_tag('kp0s1')
            eabs.append(scores_tt(sh, p, tt))
            wv_.pull(1150)
        eab_hist[j] = eabs
        queue_ctx_block(j)
        if j == 4:
            for st in range(4):
                wv_.add('w0', 1700,
                        (lambda s: lambda: wave0_st(
                            s, nc.sync if s % 2 else nc.gpsimd))(st))

    # drain everything except the last block's ctx, then the tail
    wv_.pull_tag('ctx6')
    wv_.pull_tag('w0')
    wv_.pull_tag('ctx7')   # ctx(1,3) + its normalize

    # wave1 (rows 512:1023): pair-3 deferred so these 24 matmuls run
    # during the final normalize and keep the PE warm.
    t45 = []
    for st in (4, 5):
        ps = pp_sc.tile([P, 1024], F32, name="po2", tag="sc")
        t45.append(ps)
        for p4 in range(3):
            for ih in range(2):
                nc.tensor.matmul(
                    ps[:, ih * 512:(ih + 1) * 512],
                    lhsT=cat[:, p4 * 1024 + st * P:p4 * 1024 + (st + 1) * P],
                    rhs=wo_t[:, p4 * 1024 + ih * 512:p4 * 1024 + (ih + 1) * 512],
                    start=(p4 == 0), stop=False)
    t6 = []
    for ih in range(2):
        ps = pp_mm.tile([P, 512], F32, name="po3", tag="mm")
        t6.append(ps)
        for p4 in range(3):
            nc.tensor.matmul(
                ps[:],
                lhsT=cat[:, p4 * 1024 + 6 * P:p4 * 1024 + 7 * P],
                rhs=wo_t[:, p4 * 1024 + ih * 512:p4 * 1024 + (ih + 1) * 512],
                start=(p4 == 0), stop=False)
    # pair-3 finishers + drains (split scalar/vector: ACT is idle now)
    for i, st in enumerate((4, 5)):
        ps = t45[i]
        for ih in range(2):
            nc.tensor.matmul(
                ps[:, ih * 512:(ih + 1) * 512],
                lhsT=cat[:, 3 * 1024 + st * P:3 * 1024 + (st + 1) * P],
                rhs=wo_t[:, 3 * 1024 + ih * 512:3 * 1024 + (ih + 1) * 512],
                start=False, stop=True)
    for ih in range(2):
        nc.tensor.matmul(
            t6[ih][:],
            lhsT=cat[:, 3 * 1024 + 6 * P:3 * 1024 + 7 * P],
            rhs=wo_t[:, 3 * 1024 + ih * 512:3 * 1024 + (ih + 1) * 512],
            start=False, stop=True)
    ob4 = obp.tile([P, 1024], BF16, name="ob4", tag="ob")
    nc.scalar.copy(ob4[:], t45[0][:])
    nc.sync.dma_start(out=out[4 * P:5 * P, :], in_=ob4[:])
    ob6 = obp.tile([P, 1024], BF16, name="ob6", tag="ob")
    nc.vector.tensor_copy(ob6[:, 0:512], t6[0][:])
    nc.vector.tensor_copy(ob6[:, 512:1024], t6[1][:])
    nc.gpsimd.dma_start(out=out[6 * P:7 * P, :], in_=ob6[:])
    ob5 = obp.tile([P, 1024], BF16, name="ob5", tag="ob")
    nc.scalar.copy(ob5[:], t45[1][:])
    nc.sync.dma_start(out=out[5 * P:6 * P, :], in_=ob5[:])
    # st7 on the freed pp_mm banks
    t7 = []
    for ih in range(2):
        ps = pp_mm.tile([P, 512], F32, name="po4", tag="mm")
        t7.append(ps)
        for p4 in range(4):
            nc.tensor.matmul(
                ps[:],
                lhsT=cat[:, p4 * 1024 + 7 * P:p4 * 1024 + 8 * P],
                rhs=wo_t[:, p4 * 1024 + ih * 512:p4 * 1024 + (ih + 1) * 512],
                start=(p4 == 0), stop=(p4 == 3))
    ob7 = obp.tile([P, 1024], BF16, name="ob7", tag="ob")
    nc.vector.tensor_copy(ob7[:, 0:512], t7[0][:])
    nc.scalar.copy(ob7[:, 512:1024], t7[1][:])
    nc.sync.dma_start(out=out[7 * P:8 * P, :], in_=ob7[:])


_CACHE = {}


def build():
    if "nc" in _CACHE:
        return _CACHE["nc"]
    nc = bacc.Bacc("TRN2", target_bir_lowering=False, debug=False)
    xqT = nc.dram_tensor("xqT", [P, NET * S], BF16, kind="ExternalInput").ap()
    xkT = nc.dram_tensor("xkT", [P, NET * S], BF16, kind="ExternalInput").ap()
    xvT = nc.dram_tensor("xvT", [P, NET * S], BF16, kind="ExternalInput").ap()
    wqT = nc.dram_tensor("wqT", [P, NET * HPC * HD], BF16, kind="ExternalInput").ap()
    wkT = nc.dram_tensor("wkT", [P, NET * HPC * HD], BF16, kind="ExternalInput").ap()
    wvT = nc.dram_tensor("wvT", [P, NET * HPC * HD], BF16, kind="ExternalInput").ap()
    woT = nc.dram_tensor("woT", [P, 4 * E], BF16, kind="ExternalInput").ap()
    out = nc.dram_tensor("out", [S, E], BF16, kind="ExternalOutput").ap()
    with tile.TileContext(nc) as tc, ExitStack() as ctx:
        _emit(nc, tc, ctx, (xqT, xkT, xvT, wqT, wkT, wvT, woT, out))
    nc.compile()
    _CACHE["nc"] = nc
    return nc


def make_in_maps(query, key, value, Wq, Wk, Wv, Wo):
    in_maps = []
    for c in range(8):
        b, g = divmod(c, 2)
        hs = slice(g * HPC, (g + 1) * HPC)

        def bf(a):
            return np.ascontiguousarray(a).astype(BF)

        def sbuf_tile(a):
            # [E_or_512, N] -> the SBUF-resident layout [128, n_et * N]:
            # row p, col et*N+c  =  a[et*128 + p, c]
            et = a.shape[0] // P
            return bf(a.reshape(et, P, -1).transpose(1, 0, 2).reshape(P, -1))

        def w_pairmajor(W):
            # [8, 64, E] -> [128, p*1024 + et*128 + (h_in_pair*64 + d)]
            W8 = np.asarray(W, np.float32)
            blocks = []
            for p in range(NPAIR):
                a = W8[2 * p:2 * p + 2].transpose(2, 0, 1).reshape(E, 2 * HD)
                blocks.append(a.reshape(NET, P, 2 * HD).transpose(1, 0, 2)
                              .reshape(P, NET * 2 * HD))
            return bf(np.concatenate(blocks, axis=1))

        in_maps.append({
            "xqT": sbuf_tile(np.asarray(query[b], np.float32).T),
            "xkT": sbuf_tile(np.asarray(key[b], np.float32).T),
            "xvT": sbuf_tile(np.asarray(value[b], np.float32).T),
            "wqT": w_pairmajor(np.asarray(Wq[hs], np.float32)),
            "wkT": w_pairmajor(np.asarray(Wk[hs], np.float32)),
            "wvT": sbuf_tile(np.asarray(Wv[hs], np.float32).transpose(2, 0, 1).reshape(E, HPC * HD)),
            "woT": sbuf_tile(np.asarray(Wo[:, g * HPC * HD:(g + 1) * HPC * HD], np.float32).T),
        })
    return in_maps


def kernel(query, key, value, Wq, Wk, Wv, Wo):
    nc = build()
    in_maps = make_in_maps(query, key, value, Wq, Wk, Wv, Wo)
    res = run_bass_kernel_spmd(nc, in_maps, list(range(8))).results
    out = np.empty((B, S, E), np.float32)
    for b in range(B):
        out[b] = (res[2 * b]["out"].astype(np.float32)
                  + res[2 * b + 1]["out"].astype(np.float32))
    return out
